# revision 35
# baseline (speedup 1.0000x reference)
"""Trainium2 Bass kernel for leave-one-out Nadaraya-Watson regression
(nn_Net_41420664602632, retrieval_knn).

Math
----
reference:
    Fx = x @ W.T ; Ft = train_X @ W.T          [N, 3]
    K[j,i,c] = exp(-((Ft[j,c]-Fx[i,c])/h)^2/2), K[i,i,c] = 0
    out[i,c] = sum_j K[j,i,c]*Y[j,c] / sum_j K[j,i,c]

With a = Ft/(sqrt(2)*h), b = Fx/(sqrt(2)*h):
    K[j,i] = exp(-(a_j-b_i)^2) = exp(-b_i^2) * g[j,i],
    g[j,i] = exp(2*a_j*b_i - a_j^2)
The exp(-b_i^2) factor is common to numerator and denominator and cancels
in the ratio, so the device only computes g and its two j-reductions.

Device program (per core, j-shard of 512 training points)
---------------------------------------------------------
for jt in 4 (j-tiles of 128), c in 3:
    g = ScalarE.activation(Exp, in=bcast(b[:,c]) [128,4096],
                           scale=2*a_j (per-partition), bias=-a_j^2)
    for ic in 8: PE matmul [Y_j,1]^T @ g[:, ic*512:...] -> PSUM[32c:32c+2]
        (fp32, col-tiled at partition offsets 0/32/64, accumulated over jt)
Host sums the 8 cores' [3,2,4096] partials, subtracts the j==i self term,
and divides.
"""

import os

import numpy as np

import concourse.bass as bass
import concourse.tile as tile
from concourse import bacc, mybir
from concourse.bass_utils import run_bass_kernel_spmd

N = 4096       # training/query points
C = 3          # projected channels (fc1 out_features)
NCORES = 8
JSH = N // NCORES        # 512: j-shard per core
JTILES = JSH // 128      # 4
ICH = 512                # moving free-dim chunk = one PSUM bank
NIC = N // ICH           # 8

# bb materialization: "dma" = broadcast DMA from DRAM row (stride-0 source),
# "gpsimd" = DMA row to SBUF then GpSimd partition_broadcast.
BB_MODE = os.environ.get("BB_MODE", "dma")
# matmul operand dtype: "f32" exact 2-pass half-speed ("f32r" is broken on
# this toolchain: known all-zero HW output for float32r weight loads)
MM_DTYPE = os.environ.get("MM_DTYPE", "f32")

_CACHE = {}


def _build_nc(n=N, ncores=NCORES, bb_mode=BB_MODE, mm_dtype=MM_DTYPE):
    key = (n, ncores, bb_mode, mm_dtype)
    if key in _CACHE:
        return _CACHE[key]
    jsh = n // ncores
    jtiles = jsh // 128
    nic = n // ICH
    f32 = mybir.dt.float32
    ncols = C * jtiles * 2
    # pad the per-partition row to >=512B so the DMA runs at line rate
    # (below 512B SDMA falls into read-modify-write per partition)
    ncols_pad = 128

    nc = bacc.Bacc("TRN2", target_bir_lowering=False, debug=False)
    # b is pre-replicated across partitions on the host: a plain contiguous
    # DMA is ~10x faster than a stride-0 broadcast DMA (which degenerates to
    # one descriptor per partition)
    bsrc = nc.dram_tensor("bsrc", [C, 128, n], f32, kind="ExternalInput")
    sb_d = nc.dram_tensor("scalebias", [128, ncols_pad], f32, kind="ExternalInput")
    st_d = nc.dram_tensor("stat", [128, ncols_pad], f32, kind="ExternalInput")
    out_d = nc.dram_tensor("out", [C, 2, n], f32, kind="ExternalOutput")

    with tile.TileContext(nc) as tc:
        with (
            tc.tile_pool(name="const", bufs=1) as constp,
            tc.tile_pool(name="bb", bufs=1) as bbp,
            tc.tile_pool(name="g", bufs=4) as gp,
            tc.tile_pool(name="outsb", bufs=1) as outp,
            tc.tile_pool(name="psum", bufs=1, space=bass.MemorySpace.PSUM) as pp,
        ):
            # warm the ACT exp table set immediately (overlaps input upload);
            # reads a framework const so it has no dependencies at all
            warm = constp.tile([128, 1], f32, tag="warm")
            zero_ap = nc.const_aps.scalar_like(0.0, warm[:])
            nc.scalar.activation(warm[:], zero_ap, mybir.ActivationFunctionType.Exp)

            mmdt = mybir.dt.float32r if mm_dtype == "f32r" else f32

            # channel 0's bb first: it gates the first ACTIVATE
            bbs = []
            for c in range(C):
                bbs.append(
                    bbp.tile([128, n], f32, name=f"bb{c}", tag=f"bb{c}")
                )

            def load_bb(c, nch, nfree=1):
                # chunk by partitions: each DMA moves [128/nch, n/nfree] with
                # 16KB-contiguous rows (full descriptors, line rate)
                p = 128 // nch
                w = n // nfree
                for h in range(nfree):
                    for q in range(nch):
                        nc.sync.dma_start(
                            bbs[c][q * p : (q + 1) * p, h * w : (h + 1) * w],
                            bsrc.ap()[c][q * p : (q + 1) * p, h * w : (h + 1) * w],
                        )

            # bb0 in free-halves so the (split) first ACT starts on half 0
            load_bb(0, 4, nfree=2)

            sb = constp.tile([128, ncols_pad], f32, tag="sb")
            st = constp.tile([128, ncols_pad], mmdt, tag="st")
            nc.sync.dma_start(sb[:], sb_d.ap())
            nc.sync.dma_start(st[:], st_d.ap().bitcast(mmdt))

            load_bb(1, 4)
            load_bb(2, 4)

            acc = pp.tile([128, n], f32, tag="acc")

            # num/den pairs land at outsb[0:2, c*n + ic*ICH : ...]
            outsb = outp.tile([2, C * n], f32, tag="outsb")

            # channel-major: channel 0 computes while later broadcasts land.
            # col-group (c+ic)%4 rotates so consecutive matmuls hit distinct
            # array col-groups AND distinct PSUM banks.
            for c in range(C):
                for jt in range(jtiles):
                    g = gp.tile([128, n], mmdt, tag="g")
                    k = (c * jtiles + jt) * 2
                    # the very first op is split in halves so it can start
                    # once the first half of bb0 has landed
                    nsplit = 2 if (c == 0 and jt == 0) else 1
                    w = n // nsplit
                    for h in range(nsplit):
                        nc.scalar.activation(
                            g[:, h * w : (h + 1) * w],
                            bbs[c][:, h * w : (h + 1) * w],
                            mybir.ActivationFunctionType.Exp,
                            bias=sb[:, k + 1 : k + 2],
                            scale=sb[:, k : k + 1],
                        )
                    for ic in range(nic):
                        grp = 32 * ((c + ic) % 4)
                        nc.tensor.matmul(
                            acc[grp : grp + 2, ic * ICH : (ic + 1) * ICH],
                            lhsT=st[:, k : k + 2],
                            rhs=g[:, ic * ICH : (ic + 1) * ICH],
                            start=(jt == 0),
                            stop=(jt == jtiles - 1),
                            tile_position=(0, grp),
                        )
                # evacuate each slot as its accumulation stops; overlaps the
                # next channel's compute. Alternate DVE/ACT so the last
                # channel's evacuation drains twice as fast.
                for ic in range(nic):
                    grp = 32 * ((c + ic) % 4)
                    dst = outsb[:, c * n + ic * ICH : c * n + (ic + 1) * ICH]
                    src = acc[grp : grp + 2, ic * ICH : (ic + 1) * ICH]
                    # ScalarE only helps on the last channel, after its
                    # ACTIVATE stream is done — otherwise it delays it
                    if c == C - 1 and ic % 2 == 1:
                        nc.scalar.copy(dst, src)
                    else:
                        nc.vector.tensor_copy(dst, src)
            for c in range(C):
                nc.sync.dma_start(
                    out_d.ap()[c], outsb[:, c * n : (c + 1) * n]
                )

    nc.compile()
    _CACHE[key] = nc
    return nc


def _prep_inputs(x, train_X, Y, W, h, n=N, ncores=NCORES):
    """Host-side prep: projections + per-core input maps (all float32)."""
    jsh = n // ncores
    jtiles = jsh // 128
    ncols = C * jtiles * 2
    x64 = np.asarray(x, np.float64)
    t64 = np.asarray(train_X, np.float64)
    W64 = np.asarray(W, np.float64)
    hv = float(np.asarray(h).reshape(-1)[0])
    s = 1.0 / (np.sqrt(2.0) * hv)
    b = (x64 @ W64.T) * s          # queries   [n, C]
    a = (t64 @ W64.T) * s          # training  [n, C]
    a32 = a.astype(np.float32).astype(np.float64)  # device sees fp32 values
    b32 = b.astype(np.float32).astype(np.float64)

    Yf = np.asarray(Y, np.float64)
    # [C, 128, n]: b replicated across partitions (same array shared by all
    # cores' input maps — only built once)
    bsrc = np.ascontiguousarray(
        np.broadcast_to(
            b32.T.astype(np.float32)[:, None, :], (C, 128, b32.shape[0])
        )
    )

    in_maps = []
    for r in range(ncores):
        j0 = r * jsh
        sbm = np.zeros((128, 128), np.float32)
        stm = np.zeros((128, 128), np.float32)
        for c in range(C):
            for jt in range(jtiles):
                k = (c * jtiles + jt) * 2
                aj = a32[j0 + jt * 128 : j0 + (jt + 1) * 128, c]
                sbm[:, k] = (2.0 * aj).astype(np.float32)
                sbm[:, k + 1] = (-(aj * aj)).astype(np.float32)
                stm[:, k] = Yf[j0 + jt * 128 : j0 + (jt + 1) * 128, c].astype(
                    np.float32
                )
                stm[:, k + 1] = 1.0
        in_maps.append({"bsrc": bsrc, "scalebias": sbm, "stat": stm})
    return in_maps, a32, b32


def _combine(results, Y, a32, b32, n=N):
    """Sum per-core partials, subtract self term, divide. float64 on host."""
    num = np.zeros((n, C), np.float64)
    den = np.zeros((n, C), np.float64)
    for res in results:
        o = np.asarray(res["out"], np.float64)  # [C, 2, n]
        num += o[:, 0, :].T
        den += o[:, 1, :].T
    # leave-one-out: remove the j == i term g_ii = exp(2 a_i b_i - a_i^2)
    g_self = np.exp(
        np.float32(2.0) * a32.astype(np.float32) * b32.astype(np.float32)
        - np.square(a32.astype(np.float32)),
        dtype=np.float32,
    ).astype(np.float64)
    num -= g_self * np.asarray(Y, np.float64)
    den -= g_self
    return (num / den).astype(np.float32)


def kernel(x, train_X, Y, W, h):
    nc = _build_nc()
    in_maps, a32, b32 = _prep_inputs(x, train_X, Y, W, h)
    res = run_bass_kernel_spmd(nc, in_maps, core_ids=list(range(NCORES)))
    return _combine(res.results, Y, a32, b32)
